# revision 10
# baseline (speedup 1.0000x reference)
"""Haar DWT (2x2 block transform) for Trainium2, data-parallel over 8 NeuronCores.

Full input x: (16, 64, 256, 256) fp32 -> output (16, 256, 128, 128) fp32 where
out[b, 4c+k] = subband k of channel c, k in [cA, cH, cV, cD].

Sharding: batch dim 16 -> 2 per core. Per core the (2, 64) batch/channel dims
flatten to exactly 128 images = the SBUF partition dim; each partition owns one
256x256 image laid out contiguously in its free dim.

Per-core pipeline (per 32-row tile of every image):
  1. DMA in  (128, 8192) fp32               [nc.sync / HWDGE, 4 MiB contiguous]
  2. ScalarE: x *= 0.5 in place             [folds the Haar 1/2 scale]
  3. VectorE: u = top+bot, v = bot-top      [vertical butterfly, unit stride]
  4. VectorE: even+odd -> [cA|cH], odd-even -> [cV|cD]   [horizontal butterfly]
  5. DMA out (128, 4x2048) to the 4 subband regions in one strided store
"""

import numpy as np

B, C, H, W = 16, 64, 256, 256
N_CORES = 8
B_PER = B // N_CORES  # 2
IMGS = B_PER * C  # 128 images/core = SBUF partitions
IMG_PIX = H * W  # 65536 elements per image
# tapered schedule: small tiles at the start (quick pipeline fill) and the
# end (short final out-DMA + drain), big tiles in the middle for efficiency
TILE_ROWS = [16, 16, 32, 32, 32, 32, 32, 32, 16, 8, 8]
assert sum(TILE_ROWS) == H
MAX_K = max(TILE_ROWS) * W  # slot size for the tile pools
SUB = (H // 2) * (W // 2)  # 16384 elements per subband

_CACHE: dict = {}


def build_nc():
    import concourse.bacc as bacc
    import concourse.mybir as mybir
    from concourse.tile import TileContext

    fp32 = mybir.dt.float32
    # Bacc (not plain Bass): its generate_event_semaphores pass splits
    # multi-sem waits, which the TRN2 static-DMA encoding can't hold.
    nc = bacc.Bacc(target_bir_lowering=False, debug=False)
    x = nc.dram_tensor("x", [IMGS, IMG_PIX], fp32, kind="ExternalInput")
    y = nc.dram_tensor("y", [IMGS, 4 * SUB], fp32, kind="ExternalOutput")
    # y viewed per subband: (128, 4, 16384)
    y_sub = y[:].rearrange("p (k s) -> p k s", k=4)

    with TileContext(nc) as tc:
        with (
            tc.tile_pool(name="xt", bufs=2) as pool_x,
            tc.tile_pool(name="uv", bufs=2) as pool_uv,
            tc.tile_pool(name="res", bufs=2) as pool_res,
        ):
            row0 = 0
            for rows in TILE_ROWS:
                K = rows * W  # free elems / partition this tile
                out_k = K // 4  # out elems / subband / partition this tile
                xt = pool_x.tile([IMGS, MAX_K], fp32)
                nc.sync.dma_start(
                    out=xt[:, 0:K], in_=x[:, row0 * W : row0 * W + K]
                )

                # vertical butterfly: row pairs (2i, 2i+1), unit-stride operands
                xv = xt[:, 0:K].rearrange("p (i w) -> p i w", w=2 * W)
                top = xv[:, :, 0:W]
                bot = xv[:, :, W : 2 * W]
                uv = pool_uv.tile([IMGS, MAX_K], fp32)
                u = uv[:, 0 : K // 2].rearrange("p (i w) -> p i w", w=W)
                v = uv[:, K // 2 : K].rearrange("p (i w) -> p i w", w=W)
                nc.vector.tensor_add(out=u, in0=top, in1=bot)  # a+c, b+d
                # the subtract runs on GpSimd (otherwise idle) to shave DVE time
                nc.gpsimd.tensor_sub(out=v, in0=bot, in1=top)  # c-a, d-b
                # fold the Haar 1/2 on ScalarE, keeping DMAs single-dependency:
                # xt is only ever read by DVE, res only written by DVE.
                nc.scalar.mul(uv[:, 0:K], uv[:, 0:K], 0.5)

                # horizontal butterfly: column pairs; same op serves both halves
                uvp = uv[:, 0:K].rearrange("p (n u) -> p n u", u=2)
                even = uvp[:, :, 0]
                odd = uvp[:, :, 1]
                res = pool_res.tile([IMGS, MAX_K], fp32)
                nc.vector.tensor_add(out=res[:, 0 : K // 2], in0=even, in1=odd)  # [cA|cH]
                nc.vector.tensor_sub(out=res[:, K // 2 : K], in0=odd, in1=even)  # [cV|cD]

                # res = [cA|cH|cV|cD]; one strided store to all 4 subband regions
                o0 = (row0 // 2) * (W // 2)  # out offset within each subband
                dst = y_sub[:, :, o0 : o0 + out_k]  # (128, 4, out_k)
                src = res[:, 0:K].rearrange("p (k o) -> p k o", k=4)
                nc.sync.dma_start(out=dst, in_=src)
                row0 += rows
    # run Bacc's pass pipeline (regalloc, DCE, event-semaphore wait splitting)
    nc.compile()
    return nc


def _get_nc():
    if "nc" not in _CACHE:
        _CACHE["nc"] = build_nc()
    return _CACHE["nc"]


def kernel(x: np.ndarray) -> np.ndarray:
    from concourse.bass_utils import run_bass_kernel_spmd

    x = np.ascontiguousarray(np.asarray(x), dtype=np.float32)
    assert x.shape == (B, C, H, W), x.shape

    nc = _get_nc()
    in_maps = [
        {"x": x[c * B_PER : (c + 1) * B_PER].reshape(IMGS, IMG_PIX)}
        for c in range(N_CORES)
    ]
    results = run_bass_kernel_spmd(nc, in_maps, core_ids=list(range(N_CORES))).results
    out = np.concatenate(
        [r["y"].reshape(B_PER, C * 4, H // 2, W // 2) for r in results], axis=0
    )
    return out


# revision 12
# speedup vs baseline: 1.1956x; 1.1956x over previous
"""Haar DWT (2x2 block transform) for Trainium2, data-parallel over 8 NeuronCores.

Full input x: (16, 64, 256, 256) fp32 -> output (16, 256, 128, 128) fp32 where
out[b, 4c+k] = subband k of channel c, k in [cA, cH, cV, cD].

Sharding: batch dim 16 -> 2 per core. Per core the (2, 64) batch/channel dims
flatten to exactly 128 images = the SBUF partition dim; each partition owns one
256x256 image laid out contiguously in its free dim.

Per-core pipeline (per 32-row tile of every image):
  1. DMA in  (128, 8192) fp32               [nc.sync / HWDGE, 4 MiB contiguous]
  2. ScalarE: x *= 0.5 in place             [folds the Haar 1/2 scale]
  3. VectorE: u = top+bot, v = bot-top      [vertical butterfly, unit stride]
  4. VectorE: even+odd -> [cA|cH], odd-even -> [cV|cD]   [horizontal butterfly]
  5. DMA out (128, 4x2048) to the 4 subband regions in one strided store
"""

import numpy as np

B, C, H, W = 16, 64, 256, 256
N_CORES = 8
B_PER = B // N_CORES  # 2
IMGS = B_PER * C  # 128 images/core = SBUF partitions
IMG_PIX = H * W  # 65536 elements per image
# uniform 32-row tiles measured fastest on HW (tapered start/end schedules
# added sem overhead that outweighed the shorter ramp/tail)
TILE_ROWS = [32, 32, 32, 32, 32, 32, 32, 32]
assert sum(TILE_ROWS) == H
MAX_K = max(TILE_ROWS) * W  # slot size for the tile pools
SUB = (H // 2) * (W // 2)  # 16384 elements per subband

_CACHE: dict = {}


def build_nc():
    import concourse.bacc as bacc
    import concourse.mybir as mybir
    from concourse.tile import TileContext

    fp32 = mybir.dt.float32
    # Bacc (not plain Bass): its generate_event_semaphores pass splits
    # multi-sem waits, which the TRN2 static-DMA encoding can't hold.
    nc = bacc.Bacc(target_bir_lowering=False, debug=False)
    x = nc.dram_tensor("x", [IMGS, IMG_PIX], fp32, kind="ExternalInput")
    y = nc.dram_tensor("y", [IMGS, 4 * SUB], fp32, kind="ExternalOutput")
    # y viewed per subband: (128, 4, 16384)
    y_sub = y[:].rearrange("p (k s) -> p k s", k=4)

    with TileContext(nc) as tc:
        with (
            tc.tile_pool(name="xt", bufs=2) as pool_x,
            tc.tile_pool(name="uv", bufs=2) as pool_uv,
            tc.tile_pool(name="res", bufs=2) as pool_res,
        ):
            row0 = 0
            for rows in TILE_ROWS:
                K = rows * W  # free elems / partition this tile
                out_k = K // 4  # out elems / subband / partition this tile
                xt = pool_x.tile([IMGS, MAX_K], fp32)
                nc.sync.dma_start(
                    out=xt[:, 0:K], in_=x[:, row0 * W : row0 * W + K]
                )

                # vertical butterfly: row pairs (2i, 2i+1), unit-stride operands
                xv = xt[:, 0:K].rearrange("p (i w) -> p i w", w=2 * W)
                top = xv[:, :, 0:W]
                bot = xv[:, :, W : 2 * W]
                uv = pool_uv.tile([IMGS, MAX_K], fp32)
                u = uv[:, 0 : K // 2].rearrange("p (i w) -> p i w", w=W)
                v = uv[:, K // 2 : K].rearrange("p (i w) -> p i w", w=W)
                nc.vector.tensor_add(out=u, in0=top, in1=bot)  # a+c, b+d
                nc.vector.tensor_sub(out=v, in0=bot, in1=top)  # c-a, d-b
                # fold the Haar 1/2 on ScalarE, keeping DMAs single-dependency:
                # xt is only ever read by DVE, res only written by DVE.
                nc.scalar.mul(uv[:, 0:K], uv[:, 0:K], 0.5)

                # horizontal butterfly: column pairs; same op serves both halves
                uvp = uv[:, 0:K].rearrange("p (n u) -> p n u", u=2)
                even = uvp[:, :, 0]
                odd = uvp[:, :, 1]
                res = pool_res.tile([IMGS, MAX_K], fp32)
                nc.vector.tensor_add(out=res[:, 0 : K // 2], in0=even, in1=odd)  # [cA|cH]
                nc.vector.tensor_sub(out=res[:, K // 2 : K], in0=odd, in1=even)  # [cV|cD]

                # res = [cA|cH|cV|cD]; one strided store to all 4 subband regions
                o0 = (row0 // 2) * (W // 2)  # out offset within each subband
                dst = y_sub[:, :, o0 : o0 + out_k]  # (128, 4, out_k)
                src = res[:, 0:K].rearrange("p (k o) -> p k o", k=4)
                nc.sync.dma_start(out=dst, in_=src)
                row0 += rows
    # run Bacc's pass pipeline (regalloc, DCE, event-semaphore wait splitting)
    nc.compile()
    return nc


def _get_nc():
    if "nc" not in _CACHE:
        _CACHE["nc"] = build_nc()
    return _CACHE["nc"]


def kernel(x: np.ndarray) -> np.ndarray:
    from concourse.bass_utils import run_bass_kernel_spmd

    x = np.ascontiguousarray(np.asarray(x), dtype=np.float32)
    assert x.shape == (B, C, H, W), x.shape

    nc = _get_nc()
    in_maps = [
        {"x": x[c * B_PER : (c + 1) * B_PER].reshape(IMGS, IMG_PIX)}
        for c in range(N_CORES)
    ]
    results = run_bass_kernel_spmd(nc, in_maps, core_ids=list(range(N_CORES))).results
    out = np.concatenate(
        [r["y"].reshape(B_PER, C * 4, H // 2, W // 2) for r in results], axis=0
    )
    return out


# revision 14
# speedup vs baseline: 1.2263x; 1.0257x over previous
"""Haar DWT (2x2 block transform) for Trainium2, data-parallel over 8 NeuronCores.

Full input x: (16, 64, 256, 256) fp32 -> output (16, 256, 128, 128) fp32 where
out[b, 4c+k] = subband k of channel c, k in [cA, cH, cV, cD].

Sharding: batch dim 16 -> 2 per core. Per core the (2, 64) batch/channel dims
flatten to exactly 128 images = the SBUF partition dim; each partition owns one
256x256 image laid out contiguously in its free dim.

Per-core pipeline (per 32-row tile of every image):
  1. DMA in  (128, 8192) fp32               [nc.sync / HWDGE, 4 MiB contiguous]
  2. ScalarE: x *= 0.5 in place             [folds the Haar 1/2 scale]
  3. VectorE: u = top+bot, v = bot-top      [vertical butterfly, unit stride]
  4. VectorE: even+odd -> [cA|cH], odd-even -> [cV|cD]   [horizontal butterfly]
  5. DMA out (128, 4x2048) to the 4 subband regions in one strided store
"""

import numpy as np

B, C, H, W = 16, 64, 256, 256
N_CORES = 8
B_PER = B // N_CORES  # 2
IMGS = B_PER * C  # 128 images/core = SBUF partitions
IMG_PIX = H * W  # 65536 elements per image
# uniform 32-row tiles measured fastest on HW (tapered start/end schedules
# added sem overhead that outweighed the shorter ramp/tail)
TILE_ROWS = [32, 32, 32, 32, 32, 32, 32, 32]
assert sum(TILE_ROWS) == H
MAX_K = max(TILE_ROWS) * W  # slot size for the tile pools
SUB = (H // 2) * (W // 2)  # 16384 elements per subband

_CACHE: dict = {}


def build_nc():
    import concourse.bacc as bacc
    import concourse.mybir as mybir
    from concourse.tile import TileContext

    fp32 = mybir.dt.float32
    # Bacc (not plain Bass): its generate_event_semaphores pass splits
    # multi-sem waits, which the TRN2 static-DMA encoding can't hold.
    nc = bacc.Bacc(target_bir_lowering=False, debug=False)
    x = nc.dram_tensor("x", [IMGS, IMG_PIX], fp32, kind="ExternalInput")
    y = nc.dram_tensor("y", [IMGS, 4 * SUB], fp32, kind="ExternalOutput")
    # y viewed per subband: (128, 4, 16384)
    y_sub = y[:].rearrange("p (k s) -> p k s", k=4)

    with TileContext(nc) as tc:
        with (
            tc.tile_pool(name="xt", bufs=4) as pool_x,
            tc.tile_pool(name="uv", bufs=2) as pool_uv,
        ):
            row0 = 0
            for rows in TILE_ROWS:
                K = rows * W  # free elems / partition this tile
                out_k = K // 4  # out elems / subband / partition this tile
                xt = pool_x.tile([IMGS, MAX_K], fp32)
                nc.sync.dma_start(
                    out=xt[:, 0:K], in_=x[:, row0 * W : row0 * W + K]
                )

                # vertical butterfly: row pairs (2i, 2i+1), unit-stride operands
                xv = xt[:, 0:K].rearrange("p (i w) -> p i w", w=2 * W)
                top = xv[:, :, 0:W]
                bot = xv[:, :, W : 2 * W]
                uv = pool_uv.tile([IMGS, MAX_K], fp32)
                u = uv[:, 0 : K // 2].rearrange("p (i w) -> p i w", w=W)
                v = uv[:, K // 2 : K].rearrange("p (i w) -> p i w", w=W)
                nc.vector.tensor_add(out=u, in0=top, in1=bot)  # a+c, b+d
                nc.vector.tensor_sub(out=v, in0=bot, in1=top)  # c-a, d-b
                # fold the Haar 1/2 on ScalarE, keeping DMAs single-dependency:
                # xt is only ever read by DVE, res only written by DVE.
                nc.scalar.mul(uv[:, 0:K], uv[:, 0:K], 0.5)

                # horizontal butterfly: column pairs; same op serves both halves
                uvp = uv[:, 0:K].rearrange("p (n u) -> p n u", u=2)
                even = uvp[:, :, 0]
                odd = uvp[:, :, 1]
                # pass-2 results go back into xt (its data is dead after pass 1);
                # the freed SBUF pays for deeper input prefetch (xt bufs=4)
                res = xt
                nc.vector.tensor_add(out=res[:, 0 : K // 2], in0=even, in1=odd)  # [cA|cH]
                nc.vector.tensor_sub(out=res[:, K // 2 : K], in0=odd, in1=even)  # [cV|cD]

                # res = [cA|cH|cV|cD]; one strided store to all 4 subband regions
                o0 = (row0 // 2) * (W // 2)  # out offset within each subband
                dst = y_sub[:, :, o0 : o0 + out_k]  # (128, 4, out_k)
                src = res[:, 0:K].rearrange("p (k o) -> p k o", k=4)
                nc.sync.dma_start(out=dst, in_=src)
                row0 += rows
    # run Bacc's pass pipeline (regalloc, DCE, event-semaphore wait splitting)
    nc.compile()
    return nc


def _get_nc():
    if "nc" not in _CACHE:
        _CACHE["nc"] = build_nc()
    return _CACHE["nc"]


def kernel(x: np.ndarray) -> np.ndarray:
    from concourse.bass_utils import run_bass_kernel_spmd

    x = np.ascontiguousarray(np.asarray(x), dtype=np.float32)
    assert x.shape == (B, C, H, W), x.shape

    nc = _get_nc()
    in_maps = [
        {"x": x[c * B_PER : (c + 1) * B_PER].reshape(IMGS, IMG_PIX)}
        for c in range(N_CORES)
    ]
    results = run_bass_kernel_spmd(nc, in_maps, core_ids=list(range(N_CORES))).results
    out = np.concatenate(
        [r["y"].reshape(B_PER, C * 4, H // 2, W // 2) for r in results], axis=0
    )
    return out
